# revision 4
# baseline (speedup 1.0000x reference)
"""HQQ 4-bit quantized linear on 8 Trainium2 NeuronCores (Bass/Tile).

out[4096, 11008] = x[4096, 4096] @ dequant(W_q, scale, zero).T + bias

Core c owns output columns [c*1376, (c+1)*1376) (column-parallel, x
replicated): o = g_row*172 + j, group g = j*4096 + i; core c holds
g_rows 8*(c%4)..8*(c%4)+8 of the hi (c<4) / lo (c>=4) nibble plane.

Host-side marshalling (bit/layout repack only; dequant + matmul run on
device): x.T fp16, nibbles unpacked to fp16 in [i, o] layout,
scale/zero transposed+interleaved fp16, bias row replicated.

Device per core (PE runs zero transposes):
  phase 1: W.T = (nib - zero)*scale as two broadcast fp16
      tensor_tensor passes (DVE 2x mode; ~1/3 of subs on GPSIMD with
      their muls emitted 2 k-blocks late so they never stall the
      strict-FIFO DVE queue); weight-stream DMAs are k-pair batched on
      the ScalarE HWDGE queue so they cannot head-block the SP queue's
      x/out DMAs (plain fp16 HWDGE: the SWDGE u8 cast-DMA moves at
      ~1/4 HBM rate and loses despite half the bytes).
  phase 2: token tiles in pairs, k-outer PSUM accumulation (6 banks
      live, rotating through all 8 to avoid WAR bubbles); drain = DVE
      bias-add; x prefetched one 2MB pair ahead, first pair issued
      before any dequant traffic (the intro is HBM-rate-bound).
      26 zero matmuls on the spare PSUM bank pre-warm the PE clock
      (HAM) before the first real matmul; the last pair inits PSUM via
      K=1 bias matmuls and drains with copies split over ScalarE+DVE.
"""

import numpy as np
from contextlib import ExitStack

import concourse.bacc as bacc
import concourse.bass as bass
import concourse.mybir as mybir
import concourse.tile as tile
from concourse.bass_utils import run_bass_kernel_spmd

dt = mybir.dt

TOKENS, IN_F, OUT_F, GS = 4096, 4096, 11008, 64
G = OUT_F * IN_F // GS            # 704512 quantization groups
J = G // IN_F                     # 172 groups per (g_row, i) plane
NCORES = 8
RPC = GS // NCORES                # 8 g_rows per core
O_C = RPC * J                     # 1376 output cols per core
NT = TOKENS // 128                # 32 token tiles
NK = IN_F // 128                  # 32 contraction blocks
TQ = 256                          # tokens per x-buffer chunk (1 pair)
NQ = TOKENS // TQ                 # 8 chunks
O_SPLITS = ((0, 512), (512, 512), (1024, 352))   # psum o-tiles (1 bank each)

_CACHE = {}


def _build():
    nc = bacc.Bacc("TRN2", target_bir_lowering=False, debug=False,
                   num_devices=NCORES)

    xt_d = nc.dram_tensor("xt", [IN_F, TOKENS], dt.float16,
                          kind="ExternalInput")
    nibf_d = nc.dram_tensor("nibf", [IN_F, O_C], dt.float16,
                            kind="ExternalInput")
    szt_d = nc.dram_tensor("szt", [IN_F, 2, J], dt.float16,
                           kind="ExternalInput")
    b_d = nc.dram_tensor("bias", [128, O_C], dt.float32,
                         kind="ExternalInput")
    o_d = nc.dram_tensor("out", [TOKENS, O_C], dt.float32,
                         kind="ExternalOutput")

    with ExitStack() as ctx:
        tc = ctx.enter_context(tile.TileContext(nc))
        const = ctx.enter_context(tc.tile_pool(name="const", bufs=1))
        ph1 = ctx.enter_context(tc.tile_pool(name="ph1", bufs=5))
        xpool = ctx.enter_context(tc.tile_pool(name="xpool", bufs=2))
        opool = ctx.enter_context(tc.tile_pool(name="opool", bufs=1))
        pacc = ctx.enter_context(
            tc.tile_pool(name="pacc", bufs=1, space=bass.MemorySpace.PSUM))

        biasrep = const.tile([128, O_C], dt.float32)

        # resident transposed dequantized weights: [i-part, k-block, r, j]
        WT = const.tile([128, NK, RPC, J], dt.float16)

        # x-quad prefetch on the SP (sync) DMA stream; issue the first two
        # before any dequant DMA so nothing head-blocks them.
        xbs = {}

        def fetch(q, chunks=1):
            xb = xpool.tile([128, NK, TQ], dt.float16, tag="xb",
                            name=f"xb{q % 2}")
            kc = NK // chunks
            src = xt_d[:, q * TQ:(q + 1) * TQ].rearrange(
                "(k p) t -> p k t", p=128)
            for c in range(chunks):
                nc.sync.dma_start(
                    xb[:, c * kc:(c + 1) * kc], src[:, c * kc:(c + 1) * kc])
            xbs[q] = xb

        fetch(0, chunks=4)
        fetch(1)
        nc.sync.dma_start(biasrep[:], b_d[:])
        biash = const.tile([1, O_C], dt.float16)
        nc.scalar.copy(biash[:], biasrep[0:1, :])
        ones = const.tile([1, 128], dt.float16)
        nc.vector.memset(ones[:], 1.0)
        dum = const.tile([128, 512], dt.float16)
        nc.vector.memset(dum[:], 0.0)
        warm = pacc.tile([128, 512], dt.float32, tag="a6", name="warm")
        for _ in range(26):
            nc.tensor.matmul(warm[:], dum[:, 0:128], dum[:],
                             start=True, stop=True)

        # ---- phase 1: dequant (layout already [i, o]; no transposes) ----
        # DMAs go on the ScalarE HWDGE stream: they are paced by the ph1
        # ring (wait on dequant consumption), which would head-block the
        # SP stream's xb/out DMAs.  Fetched in k-pairs to halve the
        # dma_start queue-occupancy cost.
        pend = []

        def _flush(item):
            k, d_, s_ap = item
            nc.vector.tensor_mul(
                WT[:, k], d_[:],
                s_ap.unsqueeze(1).broadcast_to((128, RPC, J)))

        for kp in range(NK // 2):
            szt = ph1.tile([128, 2, 2, J], dt.float16, tag="szt")
            nc.scalar.dma_start(
                szt[:], szt_d[kp * 256:(kp + 1) * 256].rearrange(
                    "(b p) z j -> p b z j", p=128))
            nib2 = ph1.tile([128, 2, RPC, J], dt.float16, tag="nib2")
            if kp == 0:
                for b in range(2):
                    nc.scalar.dma_start(
                        nib2[:, b],
                        nibf_d[b * 128:(b + 1) * 128, :].rearrange(
                            "p (r j) -> p r j", r=RPC))
            else:
                nc.scalar.dma_start(
                    nib2[:], nibf_d[kp * 256:(kp + 1) * 256, :].rearrange(
                        "(b p) (r j) -> p b r j", p=128, r=RPC))
            for b in range(2):
                k = kp * 2 + b
                on_gps = k >= 2 and (k % 3) == 2  # Q7 is free now (no SWDGE)
                d = ph1.tile([128, RPC, J], dt.float16, tag="d",
                             name=f"d{b}", bufs=6)
                if on_gps:
                    # GPSIMD sub is ~4x slower than DVE; copy out the
                    # scale column so the szt ring can recycle, and defer
                    # the mul two k-blocks so it doesn't stall the
                    # strict-FIFO DVE queue.
                    scol = ph1.tile([128, J], dt.float16, tag="scol",
                                    name="scol", bufs=4)
                    nc.scalar.copy(scol[:], szt[:, b, 0])
                    nc.gpsimd.tensor_sub(
                        d[:], nib2[:, b],
                        szt[:, b, 1].unsqueeze(1).broadcast_to(
                            (128, RPC, J)))
                    pend.append((k, d, scol[:]))
                else:
                    nc.vector.tensor_sub(
                        d[:], nib2[:, b],
                        szt[:, b, 1].unsqueeze(1).broadcast_to(
                            (128, RPC, J)))
                    _flush((k, d, szt[:, b, 0]))
                while pend and (pend[0][0] <= k - 2):
                    _flush(pend.pop(0))
        while pend:
            _flush(pend.pop(0))

        # ---- phase 2: stream xT, pair-wise k-outer matmul ----
        # accumulator banks: 512 fp32 = 1 PSUM bank each; rotate 6-of-8
        # per pair so the next pair starts on just-freed banks.
        npair = 0
        for q in range(NQ):
            if 2 <= q + 1 < NQ:
                fetch(q + 1)
            xb = xbs.pop(q)
            for pr in range(TQ // 256):
                accs = [[pacc.tile([128, 512], dt.float32,
                                   tag=f"a{(npair * 6 + u * 3 + p) % 8}",
                                   name=f"acc{(npair * 6 + u * 3 + p) % 8}"
                                   )[:, 0:on]
                         for p, (ob, on) in enumerate(O_SPLITS)]
                        for u in range(2)]
                npair += 1
                last = (q == NQ - 1)
                if last:
                    for u in range(2):
                        for p, (ob, on) in enumerate(O_SPLITS):
                            nc.tensor.matmul(
                                accs[u][p][:], ones[0:1, :],
                                biash[0:1, ob:ob + on],
                                start=True, stop=False)
                for k in range(NK):
                    for u in range(2):
                        ts = pr * 2 + u
                        lhsT = xb[:, k, ts * 128:(ts + 1) * 128]
                        wk = WT[:, k].rearrange("p r j -> p (r j)")
                        for p, (ob, on) in enumerate(O_SPLITS):
                            nc.tensor.matmul(
                                accs[u][p][:], lhsT, wk[:, ob:ob + on],
                                start=(k == 0 and not last),
                                stop=(k == NK - 1))
                for u in range(2):
                    t = q * (TQ // 128) + pr * 2 + u
                    for p, (ob, on) in enumerate(O_SPLITS):
                        ot = opool.tile([128, on], dt.float32, tag=f"o{u}{p}",
                                        name=f"ot{u}{p}")
                        if last and (u, p) != (1, 2):
                            nc.scalar.copy(ot[:], accs[u][p][:])
                        elif last:
                            nc.vector.tensor_copy(ot[:], accs[u][p][:])
                        else:
                            nc.vector.tensor_add(
                                ot[:], accs[u][p][:], biasrep[:, ob:ob + on])
                        nc.sync.dma_start(
                            o_d[t * 128:(t + 1) * 128, ob:ob + on], ot[:])

    nc.compile()
    return nc


def get_nc():
    if "nc" not in _CACHE:
        _CACHE["nc"] = _build()
    return _CACHE["nc"]


def make_in_maps(x, W_q, scale, zero, bias):
    x = np.ascontiguousarray(x, dtype=np.float32)
    xt = np.ascontiguousarray(x.T).astype(np.float16)
    st = np.asarray(scale, dtype=np.float32).reshape(J, IN_F).T.astype(
        np.float16)
    zt = np.asarray(zero, dtype=np.float32).reshape(J, IN_F).T.astype(
        np.float16)
    szt = np.ascontiguousarray(np.stack([st, zt], axis=1))   # [IN_F, 2, J]
    bias = np.ascontiguousarray(bias, dtype=np.float32)
    Wb = np.asarray(W_q, dtype=np.int32).astype(np.uint8)   # [32, G]
    in_maps = [None] * NCORES
    for cg in range(4):
        slab = Wb[RPC * cg:RPC * (cg + 1)]                  # [8, G]
        for half, c in ((slab >> 4, cg), (slab & 15, cg + 4)):
            nib = np.ascontiguousarray(
                half.reshape(RPC, J, IN_F).transpose(2, 0, 1)
            ).reshape(IN_F, O_C)
            in_maps[c] = {
                "xt": xt,
                "nibf": nib.astype(np.float16),
                "szt": szt,
                "bias": np.ascontiguousarray(np.broadcast_to(
                    bias[c * O_C:(c + 1) * O_C], (128, O_C))),
            }
    return in_maps


def kernel(x, W_q, scale, zero, bias):
    nc = get_nc()
    in_maps = make_in_maps(x, W_q, scale, zero, bias)
    res = run_bass_kernel_spmd(nc, in_maps, list(range(NCORES)))
    return np.concatenate(
        [res.results[c]["out"] for c in range(NCORES)], axis=1)
